# revision 1
# baseline (speedup 1.0000x reference)
"""Trainium2 kernel for NeuralDictionaryV15 (retrieval_knn, top-1 softmax dictionary).

Reference computation:
    logits = keys @ query            # [N]
    att    = softmax(logits)         # [N]
    mask   = att >= max(att)         # top-1 (ties kept)
    out    = (mask * att) @ values   # [V]

Device work (the only part that must stream big data): logits = keys @ query.
keys is 512MB; values need only the argmax row(s), gathered on host.

Sharding: keys row-sharded across 8 cores (32768 rows each). Each core runs an
identical Bass/Tile kernel: stream its shard through SBUF in multi-MB tiles,
fused multiply+row-reduce on the Vector engine (scalar_tensor_tensor with
accum_out), emit 32768 logits. Host assembles the 262144 logits (1MB), does
softmax/mask in float32 exactly like the reference, and gathers the masked
value rows.
"""

import numpy as np

N = 262144
D = 512
V = 512
NCORES = 8
NSHARD = N // NCORES          # 32768 rows per core
P = 128                       # SBUF partitions
RB = NSHARD // P              # 256 row-blocks per core

# tunables
B = 8                         # row-blocks per DMA tile (8 -> 2MB per DMA)
KBUFS = 6                     # key tile buffers
DUMMY_OUT = True              # write STT product to a [P,1] stride-0 dummy
ALT_DMA = False               # alternate sync/scalar HWDGE rings

_CACHE = {}


def _split_waits(nc):
    """Work around walrus/concourse skew: this walrus build accepts at most
    one semaphore wait per instruction, but Tile emits several. Move extra
    waits onto same-engine nops inserted just before the instruction."""
    import concourse.mybir as mybir
    import bass_rust

    cnt = 0
    for f in nc.m.functions:
        for blk in f.blocks:
            newlist = []
            for ins in blk.instructions:
                si = ins.sync_info
                waits = list(si.on_wait) if si and si.on_wait else []
                if len(waits) > 1:
                    for w in waits[:-1]:
                        nop = bass_rust.InstNoOp(name=f"{ins.name}-wsplit{cnt}")
                        cnt += 1
                        nop.engine = ins.engine
                        nop.sync_info = mybir.SyncInfo(on_wait=[w], on_update=[])
                        newlist.append(nop)
                    ins.sync_info = mybir.SyncInfo(
                        on_wait=[waits[-1]],
                        on_update=list(si.on_update) if si.on_update else [],
                    )
                newlist.append(ins)
            blk.instructions = newlist
    return cnt


def _build_nc(b=None, kbufs=None, dummy_out=None, alt_dma=None, kdt="f32"):
    import concourse.bass as bass
    import concourse.mybir as mybir
    from concourse.tile import TileContext

    b = B if b is None else b
    kbufs = KBUFS if kbufs is None else kbufs
    dummy_out = DUMMY_OUT if dummy_out is None else dummy_out
    alt_dma = ALT_DMA if alt_dma is None else alt_dma
    kdtype = mybir.dt.float32 if kdt == "f32" else mybir.dt.bfloat16

    nc = bass.Bass()
    keys = nc.declare_dram_parameter("keys", [NSHARD, D], kdtype, isOutput=False)
    query = nc.declare_dram_parameter("query", [1, D], kdtype, isOutput=False)
    logits = nc.declare_dram_parameter("logits", [P, RB], mybir.dt.float32, isOutput=True)

    # global shard row index = (i*b + bb)*P + p  ->  tile [i] laid out [p, bb, d]
    keys_r = keys.rearrange("(n b p) d -> n p b d", p=P, b=b)

    with TileContext(nc) as tc:
        with (
            tc.tile_pool(name="ktiles", bufs=kbufs) as kpool,
            tc.tile_pool(name="scratch", bufs=2) as spool,
            tc.tile_pool(name="singles", bufs=1) as singles,
        ):
            q_tile = singles.tile([P, D], kdtype)
            q_bcast = bass.AP(
                tensor=query[:].tensor,
                offset=query[:].offset,
                ap=[[0, P], [1, D]],
            )
            nc.gpsimd.dma_start(out=q_tile[:], in_=q_bcast)

            logits_sb = singles.tile([P, RB], mybir.dt.float32)

            for i in range(RB // b):
                kt = kpool.tile([P, b, D], kdtype)
                eng = nc.scalar if (alt_dma and i % 2) else nc.sync
                eng.dma_start(out=kt[:], in_=keys_r[i])
                if dummy_out:
                    sc = spool.tile([P, 1], kdtype)
                else:
                    sc = spool.tile([P, b, D], kdtype)
                for bb in range(b):
                    j = i * b + bb
                    out_ap = sc.broadcast_to((P, D)) if dummy_out else sc[:, bb, :]
                    # out = (keys * 1.0) * query ; accum_out = row-sum(out)
                    nc.vector.scalar_tensor_tensor(
                        out=out_ap,
                        in0=kt[:, bb, :],
                        scalar=1.0,
                        in1=q_tile[:],
                        op0=mybir.AluOpType.mult,
                        op1=mybir.AluOpType.mult,
                        accum_out=logits_sb[:, j : j + 1],
                    )
            nc.sync.dma_start(out=logits[:], in_=logits_sb[:])
    _split_waits(nc)
    return nc


def _get_nc():
    if "nc" not in _CACHE:
        _CACHE["nc"] = _build_nc()
    return _CACHE["nc"]


def _run_device(keys, query, trace=False, nc=None):
    """Run the per-core logits kernel on 8 cores; return full [N] f32 logits."""
    from concourse.bass_utils import run_bass_kernel_spmd

    if nc is None:
        nc = _get_nc()
    q = np.ascontiguousarray(query.reshape(1, D).astype(np.float32, copy=False))
    in_maps = [
        {
            "keys": np.ascontiguousarray(keys[c * NSHARD : (c + 1) * NSHARD]),
            "query": q,
        }
        for c in range(NCORES)
    ]
    out = run_bass_kernel_spmd(nc, in_maps, core_ids=list(range(NCORES)), trace=trace)
    # logits dram tile [p, j] holds shard row j*128 + p
    logits = np.concatenate(
        [r["logits"].T.reshape(-1) for r in out.results]
    )
    return logits, out


def _finish(logits, values):
    """Replicate the reference softmax/mask/matvec in float32 on host."""
    m = logits.max()
    e = np.exp(logits - m, dtype=np.float32)
    z = e.sum(dtype=np.float32)
    att = e / z
    amax = att.max()
    idx = np.nonzero(att >= amax)[0]
    out = (att[idx][:, None] * values[idx].astype(np.float32)).sum(axis=0)
    return out.astype(np.float32)


def kernel(query, keys, values):
    query = np.asarray(query, dtype=np.float32)
    keys = np.asarray(keys, dtype=np.float32)
    values = np.asarray(values)
    logits, _ = _run_device(keys, query, trace=False)
    return _finish(logits, values)



# revision 2
# speedup vs baseline: 1.6519x; 1.6519x over previous
"""TensorEngine fp8 matvec kernel for NeuralDictionaryV15 (retrieval_knn).

Device computes coarse logits = keys[:, keep] @ query[keep] with keys
quantized to fp8-e4m3 and pruned to the KEEP dimensions with largest
|query| (dropped dims carry ~7% of ||q||^2 at KEEP=256 -> coarse noise
sigma ~6 vs logit spread ~22.6). Host exactly recomputes every row
whose coarse logit lands within a safety window of the max (few hundred
rows) in full f32, then assembles softmax / top-1 / matvec — so neither
fp8 quantization nor dim pruning reaches the output.

Per core (shard = 32768 rows), KEEP=256:
  kt DRAM flat fp8, per tile j: [p, c, t] = keys[row0+t, perm[c*128+p]]
  qt DRAM [128, 32] fp8: qt[k, c*16] = query[perm[c*128+k]]
Per 512-row block: two accumulating matmuls (contraction 2x128) whose
PSUM output lands on partition 32*b of one bank via column tiling
(tile_position), so blocks of a group run concurrently on different PE
sub-arrays and ONE ScalarE copy drains the whole bank with 4 lanes.
Out-DMAs ride the scalar HWDGE ring so the input ring streams
uninterrupted.
"""

import numpy as np
import ml_dtypes

N = 262144
D = 512
V = 512
NCORES = 8
NSHARD = N // NCORES          # 32768 rows per core
P = 128

# tunables
KEEP = 256                    # query dims kept on device (largest |q|)
KBUFS = 4
BLK = 512                     # rows per PSUM bank
GBLK = 4                      # blocks per PSUM group tile / out-DMA
# row counts per DMA tile: small tail tiles shrink the work left after
# the last DMA byte lands
TILES = [8192] * 3 + [4096, 2048, 1024, 512, 512]
assert sum(TILES) == NSHARD

FP8 = ml_dtypes.float8_e4m3

_CACHE = {}


def _split_waits(nc):
    """This walrus build accepts at most one semaphore wait per
    instruction; move extra Tile-emitted waits onto same-engine nops."""
    import concourse.mybir as mybir
    import bass_rust

    cnt = 0
    for f in nc.m.functions:
        for blk in f.blocks:
            newlist = []
            for ins in blk.instructions:
                si = ins.sync_info
                waits = list(si.on_wait) if si and si.on_wait else []
                if len(waits) > 1:
                    for w in waits[:-1]:
                        nop = bass_rust.InstNoOp(name=f"{ins.name}-wsplit{cnt}")
                        cnt += 1
                        nop.engine = ins.engine
                        nop.sync_info = mybir.SyncInfo(on_wait=[w], on_update=[])
                        newlist.append(nop)
                    ins.sync_info = mybir.SyncInfo(
                        on_wait=[waits[-1]],
                        on_update=list(si.on_update) if si.on_update else [],
                    )
                newlist.append(ins)
            blk.instructions = newlist
    return cnt


def _build_nc(keep=None, kbufs=None, gblk=None, tiles=None):
    import concourse.bass as bass
    import concourse.mybir as mybir
    from concourse.tile import TileContext

    keep = KEEP if keep is None else keep
    kbufs = KBUFS if kbufs is None else kbufs
    gblk = GBLK if gblk is None else gblk
    tiles = TILES if tiles is None else tiles
    ki = keep // 2             # partitions per DoubleRow group
    assert ki <= P
    fp8 = mybir.dt.float8e4
    f32 = mybir.dt.float32

    nc = bass.Bass()
    kt = nc.declare_dram_parameter("kt", [1, NSHARD * keep], fp8, isOutput=False)
    qt = nc.declare_dram_parameter("qt", [ki, 32], fp8, isOutput=False)
    logits = nc.declare_dram_parameter("logits", [1, NSHARD], f32, isOutput=True)

    with TileContext(nc) as tc:
        with (
            tc.tile_pool(name="ktiles", bufs=kbufs) as kpool,
            tc.tile_pool(name="blk", bufs=4) as bpool,
            tc.tile_pool(name="psum", bufs=8 // gblk, space="PSUM") as ppool,
            tc.tile_pool(name="singles", bufs=1) as singles,
        ):
            # qt[k, jj*16] = query[perm[jj*ki + k]] (stride 16 for DoubleRow)
            q_sb = singles.tile([ki, 32], fp8)
            nc.sync.dma_start(out=q_sb[:], in_=qt[:])
            qv = q_sb[:].rearrange("p (jj s) -> p jj s", jj=2)

            row0 = 0
            for j, tw in enumerate(tiles):
                line = 2 * tw          # fp8 elems per partition line
                ktile = kpool.tile([ki, 2, tw], fp8)
                src = bass.AP(
                    tensor=kt[:].tensor,
                    offset=row0 * keep,
                    ap=[[line, ki], [1, line]],
                )
                nc.sync.dma_start(out=ktile[:], in_=src)
                for g0 in range(0, tw // BLK, gblk):
                    gw = min(gblk, tw // BLK - g0)
                    # one PSUM bank; block b lands on partition 32*b via
                    # column tiling so the copy-out uses gw lanes at once
                    ps = ppool.tile([P, BLK], f32)
                    for b in range(gw):
                        t0 = (g0 + b) * BLK
                        for c in range(2):
                            nc.tensor.matmul(
                                out=ps[32 * b : 32 * b + 1, :],
                                lhsT=qv[:, c, 0:1],
                                rhs=ktile[:, c, t0 : t0 + BLK],
                                start=(c == 0),
                                stop=(c == 1),
                                tile_position=(0, 32 * b),
                            )
                    blk_sb = bpool.tile([P, BLK], f32)
                    nc.scalar.copy(out=blk_sb[:], in_=ps[:])
                    sb_rows = blk_sb[:].rearrange("(b x) n -> b x n", x=32)[:gw, 0, :]
                    o0 = row0 + g0 * BLK
                    nc.scalar.dma_start(
                        out=logits[0:1, o0 : o0 + gw * BLK], in_=sb_rows
                    )
                row0 += tw
    _split_waits(nc)
    return nc


def _get_nc():
    if "nc" not in _CACHE:
        _CACHE["nc"] = _build_nc()
    return _CACHE["nc"]


def _perm(query, keep):
    """Kept dims: the `keep` largest |query| entries (stable order)."""
    q = np.asarray(query, dtype=np.float32).reshape(-1)
    return np.sort(np.argsort(-np.abs(q), kind="stable")[:keep])


def _prep_core(keysq, c, tiles, keep):
    """Pack core c's pruned shard into the flat kt device layout."""
    sh = keysq[c * NSHARD : (c + 1) * NSHARD]           # [NSHARD, keep] fp8
    ki = keep // 2
    parts = []
    row0 = 0
    for tw in tiles:
        blk = sh[row0 : row0 + tw]
        # [p, jj, t] = blk[t, jj*ki + p]
        a = blk.reshape(tw, 2, ki).transpose(2, 1, 0)
        parts.append(np.ascontiguousarray(a).reshape(-1))
        row0 += tw
    return np.concatenate(parts).reshape(1, NSHARD * keep)


def _run_device(keys, query, trace=False, nc=None, tiles=None, keep=None):
    from concourse.bass_utils import run_bass_kernel_spmd
    from concurrent.futures import ThreadPoolExecutor

    tiles = TILES if tiles is None else tiles
    keep = KEEP if keep is None else keep
    ki = keep // 2
    if nc is None:
        nc = _get_nc()

    perm = _perm(query, keep)
    keysq = np.ascontiguousarray(keys[:, perm]).astype(FP8)
    qf = np.asarray(query, dtype=np.float32)[perm].reshape(2, ki)
    qq = np.zeros((ki, 32), dtype=FP8)
    qq[:, 0::16] = qf.T.astype(FP8)                     # qt[k, jj*16]

    with ThreadPoolExecutor(8) as ex:
        kts = list(ex.map(lambda c: _prep_core(keysq, c, tiles, keep), range(NCORES)))
    in_maps = [{"kt": kts[c], "qt": qq} for c in range(NCORES)]
    out = run_bass_kernel_spmd(nc, in_maps, core_ids=list(range(NCORES)), trace=trace)
    logits = np.concatenate([r["logits"].reshape(-1) for r in out.results])
    return logits, out


def _finish(coarse, keys, query, values, keep=None):
    """Exact f32 recompute of the candidate window + coarse tail for Z."""
    keep = KEEP if keep is None else keep
    q = np.asarray(query, dtype=np.float32)
    perm = _perm(query, keep)
    drop_e = float(np.sum(q * q) - np.sum(q[perm] * q[perm]))
    # window: softmax-relevant 14 + 4.7 sigma for both the candidate's own
    # pruning noise and inflation of the coarse max + fp8 quant margin
    w = 14.0 + 9.4 * np.sqrt(drop_e) + 4.0
    m = coarse.max()
    cand = np.nonzero(coarse >= m - w)[0]
    exact = keys[cand].astype(np.float32) @ q
    l = coarse.copy()
    l[cand] = exact
    mtrue = l.max()
    e = np.exp(l - mtrue, dtype=np.float32)
    z = e.sum(dtype=np.float32)
    att = e / z
    amax = att.max()
    idx = np.nonzero(att >= amax)[0]
    outv = (att[idx][:, None] * values[idx].astype(np.float32)).sum(axis=0)
    return outv.astype(np.float32)


def kernel(query, keys, values):
    query = np.asarray(query, dtype=np.float32)
    keys = np.asarray(keys, dtype=np.float32)
    values = np.asarray(values)
    coarse, _ = _run_device(keys, query, trace=False)
    return _finish(coarse, keys, query, values)


# revision 3
# speedup vs baseline: 1.8642x; 1.1285x over previous
"""TensorEngine fp8 matvec kernel for NeuralDictionaryV15 (retrieval_knn).

Device computes coarse logits = keys[:, keep] @ query[keep] with keys
quantized to fp8-e4m3 and pruned to the KEEP dimensions with largest
|query| (dropped dims carry ~7% of ||q||^2 at KEEP=256 -> coarse noise
sigma ~6 vs logit spread ~22.6). Host exactly recomputes every row
whose coarse logit lands within a safety window of the max (few hundred
rows) in full f32, then assembles softmax / top-1 / matvec — so neither
fp8 quantization nor dim pruning reaches the output.

Per core (shard = 32768 rows), KEEP=256:
  kt DRAM flat fp8, per tile j: [p, c, t] = keys[row0+t, perm[c*128+p]]
  qt DRAM [128, 32] fp8: qt[k, c*16] = query[perm[c*128+k]]
Per 512-row block: two accumulating matmuls (contraction 2x128) whose
PSUM output lands on partition 32*b of one bank via column tiling
(tile_position), so blocks of a group run concurrently on different PE
sub-arrays and ONE ScalarE copy drains the whole bank with 4 lanes.
Out-DMAs ride the scalar HWDGE ring so the input ring streams
uninterrupted.
"""

import numpy as np
import ml_dtypes

N = 262144
D = 512
V = 512
NCORES = 8
NSHARD = N // NCORES          # 32768 rows per core
P = 128

# tunables
KEEP = 256                    # query dims kept on device (largest |q|)
KBUFS = 4
BLK = 512                     # rows per PSUM bank
GBLK = 4                      # blocks per PSUM group tile / out-DMA
# row counts per DMA tile: small tail tiles shrink the work left after
# the last DMA byte lands
TILES = [4096] * 8
assert sum(TILES) == NSHARD

FP8 = ml_dtypes.float8_e4m3

_CACHE = {}


def _split_waits(nc):
    """This walrus build accepts at most one semaphore wait per
    instruction; move extra Tile-emitted waits onto same-engine nops."""
    import concourse.mybir as mybir
    import bass_rust

    cnt = 0
    for f in nc.m.functions:
        for blk in f.blocks:
            newlist = []
            for ins in blk.instructions:
                si = ins.sync_info
                waits = list(si.on_wait) if si and si.on_wait else []
                if len(waits) > 1:
                    for w in waits[:-1]:
                        nop = bass_rust.InstNoOp(name=f"{ins.name}-wsplit{cnt}")
                        cnt += 1
                        nop.engine = ins.engine
                        nop.sync_info = mybir.SyncInfo(on_wait=[w], on_update=[])
                        newlist.append(nop)
                    ins.sync_info = mybir.SyncInfo(
                        on_wait=[waits[-1]],
                        on_update=list(si.on_update) if si.on_update else [],
                    )
                newlist.append(ins)
            blk.instructions = newlist
    return cnt


def _build_nc(keep=None, kbufs=None, gblk=None, tiles=None):
    import concourse.bass as bass
    import concourse.mybir as mybir
    from concourse.tile import TileContext

    keep = KEEP if keep is None else keep
    kbufs = KBUFS if kbufs is None else kbufs
    gblk = GBLK if gblk is None else gblk
    tiles = TILES if tiles is None else tiles
    ki = keep // 2             # partitions per DoubleRow group
    assert ki <= P
    fp8 = mybir.dt.float8e4
    f32 = mybir.dt.float32

    nc = bass.Bass()
    kt = nc.declare_dram_parameter("kt", [1, NSHARD * keep], fp8, isOutput=False)
    qt = nc.declare_dram_parameter("qt", [ki, 32], fp8, isOutput=False)
    logits = nc.declare_dram_parameter("logits", [1, NSHARD], f32, isOutput=True)

    with TileContext(nc) as tc:
        with (
            tc.tile_pool(name="ktiles", bufs=kbufs) as kpool,
            tc.tile_pool(name="blk", bufs=4) as bpool,
            tc.tile_pool(name="psum", bufs=8 // gblk, space="PSUM") as ppool,
            tc.tile_pool(name="singles", bufs=1) as singles,
        ):
            # qt[k, jj*16] = query[perm[jj*ki + k]] (stride 16 for DoubleRow)
            q_sb = singles.tile([ki, 32], fp8)
            nc.sync.dma_start(out=q_sb[:], in_=qt[:])
            qv = q_sb[:].rearrange("p (jj s) -> p jj s", jj=2)

            row0 = 0
            for j, tw in enumerate(tiles):
                line = 2 * tw          # fp8 elems per partition line
                ktile = kpool.tile([ki, 2, tw], fp8)
                src = bass.AP(
                    tensor=kt[:].tensor,
                    offset=row0 * keep,
                    ap=[[line, ki], [1, line]],
                )
                nc.sync.dma_start(out=ktile[:], in_=src)
                for g0 in range(0, tw // BLK, gblk):
                    gw = min(gblk, tw // BLK - g0)
                    # one PSUM bank; block b lands on partition 32*b via
                    # column tiling so the copy-out uses gw lanes at once
                    ps = ppool.tile([P, BLK], f32)
                    for b in range(gw):
                        t0 = (g0 + b) * BLK
                        for c in range(2):
                            nc.tensor.matmul(
                                out=ps[32 * b : 32 * b + 1, :],
                                lhsT=qv[:, c, 0:1],
                                rhs=ktile[:, c, t0 : t0 + BLK],
                                start=(c == 0),
                                stop=(c == 1),
                                tile_position=(0, 32 * b),
                            )
                    blk_sb = bpool.tile([P, BLK], f32)
                    nc.scalar.copy(out=blk_sb[:], in_=ps[:])
                    sb_rows = blk_sb[:].rearrange("(b x) n -> b x n", x=32)[:gw, 0, :]
                    o0 = row0 + g0 * BLK
                    nc.scalar.dma_start(
                        out=logits[0:1, o0 : o0 + gw * BLK], in_=sb_rows
                    )
                row0 += tw
    _split_waits(nc)
    return nc


def _get_nc():
    if "nc" not in _CACHE:
        _CACHE["nc"] = _build_nc()
    return _CACHE["nc"]


def _perm(query, keep):
    """Kept dims: the `keep` largest |query| entries (stable order)."""
    q = np.asarray(query, dtype=np.float32).reshape(-1)
    return np.sort(np.argsort(-np.abs(q), kind="stable")[:keep])


def _prep_core(keysq, c, tiles, keep):
    """Pack core c's pruned shard into the flat kt device layout."""
    sh = keysq[c * NSHARD : (c + 1) * NSHARD]           # [NSHARD, keep] fp8
    ki = keep // 2
    parts = []
    row0 = 0
    for tw in tiles:
        blk = sh[row0 : row0 + tw]
        # [p, jj, t] = blk[t, jj*ki + p]
        a = blk.reshape(tw, 2, ki).transpose(2, 1, 0)
        parts.append(np.ascontiguousarray(a).reshape(-1))
        row0 += tw
    return np.concatenate(parts).reshape(1, NSHARD * keep)


def _run_device(keys, query, trace=False, nc=None, tiles=None, keep=None):
    from concourse.bass_utils import run_bass_kernel_spmd
    from concurrent.futures import ThreadPoolExecutor

    tiles = TILES if tiles is None else tiles
    keep = KEEP if keep is None else keep
    ki = keep // 2
    if nc is None:
        nc = _get_nc()

    perm = _perm(query, keep)
    keysq = np.ascontiguousarray(keys[:, perm]).astype(FP8)
    qf = np.asarray(query, dtype=np.float32)[perm].reshape(2, ki)
    qq = np.zeros((ki, 32), dtype=FP8)
    qq[:, 0::16] = qf.T.astype(FP8)                     # qt[k, jj*16]

    with ThreadPoolExecutor(8) as ex:
        kts = list(ex.map(lambda c: _prep_core(keysq, c, tiles, keep), range(NCORES)))
    in_maps = [{"kt": kts[c], "qt": qq} for c in range(NCORES)]
    out = run_bass_kernel_spmd(nc, in_maps, core_ids=list(range(NCORES)), trace=trace)
    logits = np.concatenate([r["logits"].reshape(-1) for r in out.results])
    return logits, out


def _finish(coarse, keys, query, values, keep=None):
    """Exact f32 recompute of the candidate window + coarse tail for Z."""
    keep = KEEP if keep is None else keep
    q = np.asarray(query, dtype=np.float32)
    perm = _perm(query, keep)
    drop_e = float(np.sum(q * q) - np.sum(q[perm] * q[perm]))
    # window: softmax-relevant 14 + 4.7 sigma for both the candidate's own
    # pruning noise and inflation of the coarse max + fp8 quant margin
    w = 14.0 + 9.4 * np.sqrt(drop_e) + 4.0
    m = coarse.max()
    cand = np.nonzero(coarse >= m - w)[0]
    exact = keys[cand].astype(np.float32) @ q
    l = coarse.copy()
    l[cand] = exact
    mtrue = l.max()
    e = np.exp(l - mtrue, dtype=np.float32)
    z = e.sum(dtype=np.float32)
    att = e / z
    amax = att.max()
    idx = np.nonzero(att >= amax)[0]
    outv = (att[idx][:, None] * values[idx].astype(np.float32)).sum(axis=0)
    return outv.astype(np.float32)


def kernel(query, keys, values):
    query = np.asarray(query, dtype=np.float32)
    keys = np.asarray(keys, dtype=np.float32)
    values = np.asarray(values)
    coarse, _ = _run_device(keys, query, trace=False)
    return _finish(coarse, keys, query, values)
